# revision 27
# baseline (speedup 1.0000x reference)
"""Trainium2 Bass kernel for nn_MemoryRetriever (cross-attention memory retriever).

Strategy (v2):
- Host-side mask compaction: only unmasked memory tokens (~50%) are sharded
  across the 8 cores; padding keys are killed with a -1e30 exp bias.  Exact
  same math (masked keys contribute exactly zero), ~2x less device work.
- Phase 1 per core: K projection + 3D-RoPE + transposed sum-of-squares for
  RMSNorm, and V projection, for the core's key shard; K (roped,
  un-normalized) and V stay resident in SBUF.  The 1/rms factor is NOT
  multiplied into K; it is folded into the Exp activation's per-partition
  scale AP later (keys sit on partitions in the score tiles).
- Phase 2 per head: scores = K_raw^T Q (Q pre-scaled by 1/sqrt(128)), Exp
  with scale=rsqrt-AP and bias=mask-AP, then attn@V and the softmax
  denominator both accumulate in PSUM across all key tiles (start/stop
  accumulation groups) - no vector-engine accumulators.  Results DMA
  straight from PSUM to DRAM in a query-block-major layout.
- One ReduceScatter (query axis) replaces the baseline AllReduce: each core
  receives exactly its 64-query slice of the summed numerators/denominators,
  normalizes, and output-projects it.  Host concatenates the 8 slices.

All matmul operands bf16 (fp32 PSUM accumulation); softmax/normalization
arithmetic fp32.
"""

import math
import sys

import numpy as np

sys.path.insert(0, "/opt/trn_rl_repo")

DIM = 1024
HEADS = 8
HD = 128
SQ = 512
N_CORES = 8
QS = SQ // N_CORES   # 64 queries per core in the tail
EPS = 1e-6
SCALE = 1.0 / math.sqrt(128.0)
NEG = -1.0e30
CHUNK_TILES = 4      # key tiles per phase-1 chunk

_cache = {}
_last_tt = None


def _build(tt=16):
    """Build + compile the per-core program for a shard of `tt` key tiles."""
    key = ("nc", tt)
    if key in _cache:
        return _cache[key]

    import concourse.bass as bass  # noqa: F401
    import concourse.tile as tile
    from concourse import mybir, bacc

    f32 = mybir.dt.float32
    bf16 = mybir.dt.bfloat16
    AF = mybir.ActivationFunctionType

    skc = tt * 128

    nc = bacc.Bacc("TRN2", target_bir_lowering=False, debug=False,
                   num_devices=N_CORES)

    def din(name, shape, dt=f32):
        return nc.dram_tensor(name, list(shape), dt, kind="ExternalInput").ap()

    # per-core sharded inputs (compacted keys)
    memT = din("memT", [DIM, skc], bf16)    # mem shard, feature-major
    ctk = din("ctk", [HD, skc])             # K rope cos table (in-head d major)
    stk = din("stk", [HD, skc])
    mbias = din("mbias", [128, tt])         # exp bias (0 valid / -1e30 pad)
    # shared inputs
    xT = din("xT", [DIM, SQ], bf16)
    wq = din("wq", [128, 8, 8, 128], bf16)  # [p,i,o,m] = Wq.T[i*128+p, o*128+m]
    wk = din("wk", [128, 8, 8, 128], bf16)
    wo = din("wo", [128, 8, 8, 128], bf16)  # [p,o,e,m] = Wo.T[o*128+p, e*128+m]
    wv = din("wv", [128, 8, DIM], bf16)     # [p,i,o] = Wv.T[i*128+p, o]
    ctq = din("ctq", [128, 8, SQ])          # q rope cos (gq*gk*SCALE folded)
    stq = din("stq", [128, 8, SQ])
    bq_t = din("bq_t", [128, 8])
    bk_t = din("bk_t", [128, 8])
    bo_t = din("bo_t", [128, 8])
    pmat = din("pmat", [128, 128], bf16)    # P.T for rope pair swap (+-1)
    ones_c = din("ones_c", [128, 1], bf16)
    eps_in = din("eps_c", [128, 1])
    eps1_in = din("eps1_c", [1, 1])

    outT = nc.dram_tensor("outT", [DIM, QS], f32, kind="ExternalOutput").ap()

    import os as _os
    _sim = _os.environ.get("KSIM", "0") == "1"
    _dbg = _os.environ.get("KDBG", "0") == "1"
    # cat: per-core partial numerators+denominators, query-block-major so a
    # ReduceScatter hands core c exactly its 64-query slice.
    # rows 0..1024: numerator (h*128+d), rows 1024..1032: denominator per head
    cat = nc.dram_tensor("cat", [N_CORES, DIM + HEADS, QS], f32)
    catrs = nc.dram_tensor("catrs", [DIM + HEADS, QS], f32)
    if _dbg:
        qdbg = nc.dram_tensor("qdbg", [128, 8, SQ], bf16, kind="ExternalOutput").ap()
        krdbg = nc.dram_tensor("krdbg", [128, 8, skc], bf16, kind="ExternalOutput").ap()
        rsbdbg = nc.dram_tensor("rsbdbg", [128, tt], f32, kind="ExternalOutput").ap()
        catdbg = nc.dram_tensor("catdbg", [N_CORES, DIM + HEADS, QS], f32, kind="ExternalOutput").ap()
        nscdbg = nc.dram_tensor("nscdbg", [128, 8, QS], f32, kind="ExternalOutput").ap()

    with tile.TileContext(nc) as tc:
        ctx_pools = []

        def pool(name, bufs, space=None):
            kw = dict(name=name, bufs=bufs)
            if space:
                kw["space"] = space
            p = tc.tile_pool(**kw)
            ctx_pools.append(p)
            return p.__enter__()

        consts = pool("consts", 1)
        resid = pool("resid", 1)
        pp = pool("pp", 3, space="PSUM")
        pp_acc = pool("pp_acc", 2, space="PSUM")
        pp_den = pool("pp_den", 1, space="PSUM")
        pp_sq = pool("pp_sq", 2, space="PSUM")

        # ---- constants / resident tensors (xT first: it gates the Q matmuls) ----
        xt_s = consts.tile([128, 8, SQ], bf16)
        nc.sync.dma_start(xt_s[:], xT.rearrange("(i p) q -> p i q", p=128))
        pt_s = consts.tile([128, 128], bf16)
        nc.sync.dma_start(pt_s[:], pmat)
        ones_s = consts.tile([128, 1], bf16)
        nc.sync.dma_start(ones_s[:], ones_c)
        bq_s = consts.tile([128, 8], f32)
        nc.sync.dma_start(bq_s[:], bq_t)
        eps1_s = consts.tile([1, 1], f32)
        nc.sync.dma_start(eps1_s[:], eps1_in)

        qT = resid.tile([128, 8, SQ], bf16)      # rope'd+normalized Q (pre-scaled)
        kr = resid.tile([128, 8, skc], bf16)     # rope'd UN-normalized K
        v_sb = resid.tile([128, tt, DIM], bf16)  # V, token-major
        rsb_all = resid.tile([128, tt], f32)     # 1/rms per key, tile-column layout
        mb_s = resid.tile([128, tt], f32)
        bk_s = consts.tile([128, 8], f32)
        eps_s = consts.tile([128, 1], f32)

        # =========== Q phase ===========
        qpool_cm = tc.tile_pool(name="qpool", bufs=1)
        qpool = qpool_cm.__enter__()
        qpf_cm = tc.tile_pool(name="qpf", bufs=2)
        qpf = qpf_cm.__enter__()
        yq = qpool.tile([128, 8, SQ], f32, tag="yq")
        ybq = qpool.tile([128, 8, SQ], bf16, tag="ybq")
        ps_sq_q = pp_den.tile([1, SQ], f32, tag="den")
        for o in range(8):
            wq_o = qpf.tile([128, 8, 128], bf16, tag="wq_o")
            nc.sync.dma_start(wq_o[:], wq[:, :, o, :])
            ps_q = pp.tile([128, SQ], f32, tag="ps")
            for i in range(8):
                nc.tensor.matmul(ps_q[:], wq_o[:, i, :], xt_s[:, i, :],
                                 start=(i == 0), stop=(i == 7))
            nc.scalar.activation(yq[:, o, :], ps_q[:], AF.Identity,
                                 bias=bq_s[:, o:o + 1])
            nc.vector.tensor_copy(ybq[:, o, :], yq[:, o, :])
            ysq = qpool.tile([128, SQ], bf16, tag="ysq")
            nc.vector.tensor_mul(ysq[:], ybq[:, o, :], ybq[:, o, :])
            nc.tensor.matmul(ps_sq_q[:], ones_s[:], ysq[:],
                             start=(o == 0), stop=(o == 7))
        # deferred const loads (off the critical first DMAs)
        nc.sync.dma_start(bk_s[:], bk_t)
        nc.sync.dma_start(eps_s[:], eps_in)
        nc.sync.dma_start(mb_s[:], mbias)
        sq_q = qpool.tile([1, SQ], f32, tag="sqr")
        nc.scalar.activation(sq_q[:], ps_sq_q[:], AF.Sqrt,
                             bias=eps1_s[:], scale=1.0 / DIM)
        rs_q = qpool.tile([1, SQ], f32, tag="rs")
        nc.vector.reciprocal(rs_q[:], sq_q[:])
        rsb_q = qpool.tile([128, SQ], f32, tag="rsb")
        nc.gpsimd.partition_broadcast(rsb_q[:], rs_q[:])
        for o in range(8):
            ctq_o = qpf.tile([128, SQ], f32, tag="ctq_o")
            nc.scalar.dma_start(ctq_o[:], ctq[:, o, :])
            stq_o = qpf.tile([128, SQ], f32, tag="stq_o")
            nc.scalar.dma_start(stq_o[:], stq[:, o, :])
            ps_sw = pp.tile([128, SQ], f32, tag="ps")
            nc.tensor.matmul(ps_sw[:], pt_s[:], ybq[:, o, :])
            t1 = qpool.tile([128, SQ], f32, tag="t1")
            nc.vector.tensor_mul(t1[:], yq[:, o, :], ctq_o[:])
            t2 = qpool.tile([128, SQ], f32, tag="t2")
            nc.vector.tensor_mul(t2[:], ps_sw[:], stq_o[:])
            nc.vector.tensor_add(t1[:], t1[:], t2[:])
            nc.vector.tensor_mul(qT[:, o, :], t1[:], rsb_q[:])
        qpf_cm.__exit__(None, None, None)
        qpool_cm.__exit__(None, None, None)

        # =========== phase 1: K rope + V for all key tiles ===========
        wpool = pool("wpool", 1)
        wk_s = wpool.tile([128, 8, 8, 128], bf16)
        nc.sync.dma_start(wk_s[:], wk)
        wv_s = wpool.tile([128, 8, DIM], bf16)
        nc.sync.dma_start(wv_s[:], wv)
        wo_s = wpool.tile([128, 8, 8, 128], bf16)
        nc.sync.dma_start(wo_s[:], wo)
        bo_s = consts.tile([128, 8], f32)
        nc.sync.dma_start(bo_s[:], bo_t)

        kpool_cm = tc.tile_pool(name="kpool", bufs=2)
        kpool = kpool_cm.__enter__()

        def make_sumsq(ct0, ntt, ysq_all):
            # deferred: emitted after the NEXT chunk's K projection so the PE
            # never waits on this chunk's ysq (DVE) completion
            def emit():
                pst = pp_sq.tile([128, ntt], f32, tag="pst")
                for ti in range(ntt):
                    for o in range(8):
                        nc.tensor.matmul(
                            pst[:, ti:ti + 1],
                            ysq_all[:, o, ti * 128:(ti + 1) * 128], ones_s[:],
                            start=(o == 0), stop=(o == 7))
                sq_t = kpool.tile([128, ntt], f32, tag="sqt")
                nc.scalar.activation(sq_t[:], pst[:], AF.Sqrt,
                                     bias=eps_s[:], scale=1.0 / DIM)
                nc.vector.reciprocal(rsb_all[:, ct0:ct0 + ntt], sq_t[:])
            return emit

        pend_sumsq = None
        for ct0 in range(0, tt, CHUNK_TILES):
            ntt = min(CHUNK_TILES, tt - ct0)
            cw = ntt * 128
            c0 = ct0 * 128
            memt = kpool.tile([128, 8, cw], bf16, tag="memt")
            nc.sync.dma_start(
                memt[:], memT[:, c0:c0 + cw].rearrange("(i p) t -> p i t", p=128))
            ctk_t = kpool.tile([128, cw], f32, tag="ctk")
            nc.sync.dma_start(ctk_t[:], ctk[:, c0:c0 + cw])
            stk_t = kpool.tile([128, cw], f32, tag="stk")
            nc.sync.dma_start(stk_t[:], stk[:, c0:c0 + cw])

            yk = kpool.tile([128, 8, cw], bf16, tag="yk")
            ysq_all = kpool.tile([128, 8, cw], bf16, tag="ysq")
            t1_tiles = {}

            def rope_finish(op, ps_sw, c0=c0, cw=cw, stk_t=stk_t, kpool=kpool,
                            t1_tiles=t1_tiles):
                # kr[op] = yk[op]*cos + (P yk[op])*sin   (1/rms deferred to Exp)
                t2 = kpool.tile([128, cw], f32, tag="t2")
                nc.vector.tensor_mul(t2[:], ps_sw[:], stk_t[:])
                nc.vector.tensor_add(kr[:, op, c0:c0 + cw],
                                     t1_tiles.pop(op)[:], t2[:])

            sw_prev = None
            for o in range(8):
                ps_y = pp.tile([128, cw], f32, tag="ps")
                for i in range(8):
                    nc.tensor.matmul(ps_y[:], wk_s[:, i, o, :], memt[:, i, :],
                                     start=(i == 0), stop=(i == 7))
                if sw_prev is not None:
                    # swap matmul for o-1 (after o's projection, so the PE
                    # never waits on yk[o-1]'s activation copy)
                    ps_sw = pp.tile([128, cw], f32, tag="ps")
                    nc.tensor.matmul(ps_sw[:], pt_s[:], yk[:, sw_prev, :])
                    rope_pend = (sw_prev, ps_sw)
                else:
                    rope_pend = None
                nc.scalar.activation(yk[:, o, :], ps_y[:], AF.Identity,
                                     bias=bk_s[:, o:o + 1])
                nc.vector.tensor_mul(ysq_all[:, o, :], yk[:, o, :], yk[:, o, :])
                t1 = kpool.tile([128, cw], f32, tag=f"t1_{o % 2}")
                nc.vector.tensor_mul(t1[:], yk[:, o, :], ctk_t[:])
                t1_tiles[o] = t1
                if rope_pend is not None:
                    rope_finish(*rope_pend)
                sw_prev = o
            ps_sw = pp.tile([128, cw], f32, tag="ps")
            nc.tensor.matmul(ps_sw[:], pt_s[:], yk[:, 7, :])
            rope_finish(7, ps_sw)

            if pend_sumsq is not None:
                pend_sumsq()

            # V projection (token-major output)
            for ti in range(ntt):
                for oh in range(2):
                    ps_v = pp.tile([128, 512], f32, tag="ps")
                    for i in range(8):
                        nc.tensor.matmul(
                            ps_v[:], memt[:, i, ti * 128:(ti + 1) * 128],
                            wv_s[:, i, oh * 512:(oh + 1) * 512],
                            start=(i == 0), stop=(i == 7))
                    nc.scalar.activation(
                        v_sb[:, ct0 + ti, oh * 512:(oh + 1) * 512], ps_v[:],
                        AF.Identity)

            pend_sumsq = make_sumsq(ct0, ntt, ysq_all)
        pend_sumsq()
        kpool_cm.__exit__(None, None, None)

        # =========== phase 2: per-head attention, PSUM accumulation ===========
        ppool_cm = tc.tile_pool(name="ppool", bufs=3)
        ppool = ppool_cm.__enter__()
        LOOK = 2
        for h in range(8):
            ps_n = pp_acc.tile([128, SQ], f32, tag="acc")
            den_d = pp_den.tile([1, SQ], f32, tag="den")
            pts = {}

            def consume(t, h=h, ps_n=ps_n, den_d=den_d, pts=pts):
                nc.tensor.matmul(ps_n[:], v_sb[:, t, h * 128:(h + 1) * 128],
                                 pts[t][:], start=(t == 0), stop=(t == tt - 1))
                nc.tensor.matmul(den_d[:], ones_s[:], pts[t][:],
                                 start=(t == 0), stop=(t == tt - 1))
                del pts[t]

            for t in range(tt):
                ps_s = pp.tile([128, SQ], f32, tag="ps")
                nc.tensor.matmul(ps_s[:], kr[:, h, t * 128:(t + 1) * 128],
                                 qT[:, h, :])
                pt = ppool.tile([128, SQ], bf16, tag="pt")
                nc.scalar.activation(pt[:], ps_s[:], AF.Exp,
                                     bias=mb_s[:, t:t + 1],
                                     scale=rsb_all[:, t:t + 1])
                pts[t] = pt
                if t >= LOOK:
                    consume(t - LOOK)
            for t in range(max(0, tt - LOOK), tt):
                consume(t)

            nsb = ppool.tile([128, SQ], f32, tag="nsb")
            nc.vector.tensor_copy(nsb[:], ps_n[:])
            dsb = ppool.tile([1, SQ], f32, tag="dsb")
            nc.vector.tensor_copy(dsb[:], den_d[:])
            nc.scalar.dma_start(
                cat[:, h * 128:(h + 1) * 128, :].rearrange("b p q -> p b q"),
                nsb[:])
            nc.gpsimd.dma_start(
                cat[:, DIM + h:DIM + h + 1, :].rearrange("b o q -> o b q"),
                dsb[:])
        ppool_cm.__exit__(None, None, None)

        # =========== reduce-scatter across cores (query axis) ===========
        tail = pool("tail", 1)
        if _dbg:
            nc.sync.dma_start(qdbg, qT[:])
            nc.sync.dma_start(krdbg, kr[:])
            nc.sync.dma_start(rsbdbg, rsb_all[:])
            nc.sync.dma_start(catdbg, cat[:, :, :])
        if _sim:
            nc.gpsimd.dma_start(catrs[:], cat[0, :, :])
        else:
            nc.gpsimd.collective_compute(
                "ReduceScatter", mybir.AluOpType.add,
                replica_groups=[list(range(N_CORES))],
                ins=[cat[:]], outs=[catrs[:]])

        # =========== per-core tail: normalize + output projection ===========
        nred = tail.tile([128, 8, QS], f32)
        nc.scalar.dma_start(
            nred[:], catrs[0:DIM, :].rearrange("(h p) q -> p h q", p=128))
        dden = tail.tile([1, 8, QS], f32)
        nc.scalar.dma_start(dden[:], catrs[DIM:DIM + HEADS, :])
        rd = tail.tile([1, 8, QS], f32)
        nc.vector.reciprocal(rd[:], dden[:])
        nsc = tail.tile([128, 8, QS], bf16)
        for h in range(8):
            rdb = tail.tile([128, QS], f32, tag="rdb")
            nc.gpsimd.partition_broadcast(rdb[:], rd[0:1, h, :])
            nc.vector.tensor_mul(nsc[:, h, :], nred[:, h, :], rdb[:])
        # sequential accumulation groups per output block (interleaved groups
        # within one PSUM bank are unsafe: a later group's `start` re-marks the
        # whole 2KB zero-region, turning sibling accumulates into overwrites)
        out_sb = tail.tile([128, 8, QS], f32)
        ps_o = pp.tile([128, 8, QS], f32, tag="ps")
        for e in range(8):
            for o in range(8):
                nc.tensor.matmul(ps_o[:, e, :], wo_s[:, o, e, :], nsc[:, o, :],
                                 start=(o == 0), stop=(o == 7))
            nc.scalar.activation(out_sb[:, e, :], ps_o[:, e, :], AF.Identity,
                                 bias=bo_s[:, e:e + 1])
        if _dbg:
            nscf = tail.tile([128, 8, QS], f32)
            nc.vector.tensor_copy(nscf[:], nsc[:])
            nc.sync.dma_start(nscdbg, nscf[:])
        nc.sync.dma_start(outT.rearrange("(e p) q -> p e q", p=128), out_sb[:])

        for p in reversed(ctx_pools):
            p.__exit__(None, None, None)

    nc.compile()
    _cache[key] = nc
    return nc


def _prep(x, mem, mask, cos_q, sin_q, cos_k, sin_k,
          Wq, bq, Wk, bk, Wv, bv, Wo, bo, gq, gk):
    import ml_dtypes
    f = np.float32
    bf = ml_dtypes.bfloat16
    x = np.asarray(x, f).reshape(SQ, DIM)
    mem = np.asarray(mem, f)
    mem = mem.reshape(-1, DIM)
    sk = mem.shape[0]
    mask = np.asarray(mask).reshape(sk)
    cos_q = np.asarray(cos_q, f)
    sin_q = np.asarray(sin_q, f)
    cos_k = np.asarray(cos_k, f)
    sin_k = np.asarray(sin_k, f)
    Wq, Wk, Wv, Wo = (np.asarray(w, f) for w in (Wq, Wk, Wv, Wo))
    bq, bk, bv, bo, gq, gk = (np.asarray(v, f) for v in (bq, bk, bv, bo, gq, gk))

    assert np.all(bv == 0.0), "nonzero bv not supported by this build"
    if not np.allclose(gk, 1.0):
        gkp = gk.reshape(-1, 2)
        assert np.allclose(gkp[:, 0], gkp[:, 1]), "unsupported non-pairwise gk"

    def tile_w(WT):  # [1024,1024] (in,out of W.T) -> [p, i, o, m]
        return np.ascontiguousarray(
            WT.reshape(8, 128, 8, 128).transpose(1, 0, 2, 3)).astype(bf)

    ii = np.arange(128)
    jj = ii // 2
    partner = ii ^ 1

    # fold gq (+pairwise gk) and the 1/sqrt(HD) score scale into the q rope
    # tables; sin pairs with partner's gq
    gq_t = (gq * gk).reshape(8, 128) * SCALE
    gq_sin = (gq.reshape(8, 128)[:, partner] * gk.reshape(8, 128)) * SCALE
    cq = cos_q[:, jj].T                # [128, SQ]
    sq = sin_q[:, jj].T
    ctq = np.ascontiguousarray(
        (cq[None, :, :] * gq_t[:, :, None]).transpose(1, 0, 2)).astype(f)
    stq = np.ascontiguousarray(
        (sq[None, :, :] * gq_sin[:, :, None]).transpose(1, 0, 2)).astype(f)

    PT = np.zeros((128, 128), f)
    even = ii[ii % 2 == 0]
    PT[even + 1, even] = -1.0
    PT[even, even + 1] = 1.0

    shared = {
        "xT": np.ascontiguousarray(x.T).astype(bf),
        "wq": tile_w(Wq.T), "wk": tile_w(Wk.T), "wo": tile_w(Wo.T),
        "wv": np.ascontiguousarray(
            Wv.T.reshape(8, 128, DIM).transpose(1, 0, 2)).astype(bf),
        "ctq": ctq, "stq": stq,
        "bq_t": np.ascontiguousarray(bq.reshape(8, 128).T),
        "bk_t": np.ascontiguousarray(bk.reshape(8, 128).T),
        "bo_t": np.ascontiguousarray(bo.reshape(8, 128).T),
        "pmat": PT.astype(bf),
        "ones_c": np.ones((128, 1), bf),
        "eps_c": np.full((128, 1), EPS, f),
        "eps1_c": np.full((1, 1), EPS, f),
    }

    # ---- mask compaction: keep only unmasked keys, pad to 8*tt*128 ----
    idx = np.flatnonzero(mask)
    m = idx.size
    tt = max(1, -(-m // (N_CORES * 128)))
    skc = tt * 128
    total = N_CORES * skc

    memc = np.zeros((total, DIM), f)
    memc[:m] = mem[idx]
    ck = np.zeros((total, HD // 2), f)
    sk_ = np.zeros((total, HD // 2), f)
    ck[:m] = cos_k[idx]
    sk_[:m] = sin_k[idx]
    mb_full = np.full(total, NEG, f)
    mb_full[:m] = 0.0

    in_maps = []
    for c in range(N_CORES):
        s = slice(c * skc, (c + 1) * skc)
        mmap = dict(shared)
        mmap["memT"] = np.ascontiguousarray(memc[s].T).astype(bf)
        mmap["ctk"] = np.ascontiguousarray(ck[s][:, jj].T)
        mmap["stk"] = np.ascontiguousarray(sk_[s][:, jj].T)
        mmap["mbias"] = np.ascontiguousarray(mb_full[s].reshape(tt, 128).T)
        in_maps.append(mmap)
    return in_maps, tt


def kernel(**inputs):
    global _last_tt
    from concourse.bass_utils import run_bass_kernel_spmd
    in_maps, tt = _prep(**inputs)
    _last_tt = tt
    nc = _build(tt)
    res = run_bass_kernel_spmd(nc, in_maps, list(range(N_CORES)))
    parts = [res.results[c]["outT"].T for c in range(N_CORES)]
    out = np.concatenate(parts, axis=0)
    return out[None].astype(np.float32)


# revision 55
# speedup vs baseline: 1.2090x; 1.2090x over previous
"""Trainium2 Bass kernel for nn_MemoryRetriever (cross-attention memory retriever).

Strategy (v2):
- Host-side mask compaction: only unmasked memory tokens (~50%) are sharded
  across the 8 cores; padding keys are killed with a -1e30 exp bias.  Exact
  same math (masked keys contribute exactly zero), ~2x less device work.
- Phase 1 per core: K projection + 3D-RoPE + transposed sum-of-squares for
  RMSNorm, and V projection, for the core's key shard; K (roped,
  un-normalized) and V stay resident in SBUF.  The 1/rms factor is NOT
  multiplied into K; it is folded into the Exp activation's per-partition
  scale AP later (keys sit on partitions in the score tiles).
- Phase 2 per head: scores = K_raw^T Q (Q pre-scaled by 1/sqrt(128)), Exp
  with scale=rsqrt-AP and bias=mask-AP, then attn@V and the softmax
  denominator both accumulate in PSUM across all key tiles (start/stop
  accumulation groups) - no vector-engine accumulators.  Results DMA
  straight from PSUM to DRAM in a query-block-major layout.
- One ReduceScatter (query axis) replaces the baseline AllReduce: each core
  receives exactly its 64-query slice of the summed numerators/denominators,
  normalizes, and output-projects it.  Host concatenates the 8 slices.

All matmul operands bf16 (fp32 PSUM accumulation); softmax/normalization
arithmetic fp32.
"""

import math
import sys

import numpy as np

sys.path.insert(0, "/opt/trn_rl_repo")

DIM = 1024
HEADS = 8
HD = 128
SQ = 512
N_CORES = 8
QS = SQ // N_CORES   # 64 queries per core in the tail
EPS = 1e-6
SCALE = 1.0 / math.sqrt(128.0)
NEG = -1.0e30
CHUNK_TILES = 4      # key tiles per phase-1 chunk

_cache = {}
_last_tt = None


def _build(tt=16):
    """Build + compile the per-core program for a shard of `tt` key tiles."""
    key = ("nc", tt)
    if key in _cache:
        return _cache[key]

    import concourse.bass as bass  # noqa: F401
    import concourse.tile as tile
    from concourse import mybir, bacc

    f32 = mybir.dt.float32
    bf16 = mybir.dt.bfloat16
    AF = mybir.ActivationFunctionType

    skc = tt * 128

    nc = bacc.Bacc("TRN2", target_bir_lowering=False, debug=False,
                   num_devices=N_CORES)

    def din(name, shape, dt=f32):
        return nc.dram_tensor(name, list(shape), dt, kind="ExternalInput").ap()

    # per-core sharded inputs (compacted keys)
    memT = din("memT", [DIM, skc], bf16)    # mem shard, feature-major
    ctk = din("ctk", [HD, skc], bf16)       # K rope cos table (in-head d major)
    stk = din("stk", [HD, skc], bf16)
    mbias = din("mbias", [128, tt])         # exp bias (0 valid / -1e30 pad)
    # shared inputs
    xT = din("xT", [DIM, SQ], bf16)
    wq = din("wq", [128, 8, 8, 128], bf16)  # [p,i,o,m] = Wq.T[i*128+p, o*128+m]
    wk = din("wk", [128, 8, 8, 128], bf16)
    wo = din("wo", [128, 8, 8, 128], bf16)  # [p,o,e,m] = Wo.T[o*128+p, e*128+m]
    wv = din("wv", [128, 8, DIM], bf16)     # [p,i,o] = Wv.T[i*128+p, o]
    ctq = din("ctq", [128, 8, SQ], bf16)    # q rope cos (gq*gk*SCALE folded)
    stq = din("stq", [128, 8, SQ], bf16)
    bq_t = din("bq_t", [128, 8])
    bk_t = din("bk_t", [128, 8])
    bo_t = din("bo_t", [128, 8])
    pmat = din("pmat", [128, 128], bf16)    # P.T for rope pair swap (+-1)
    ones_c = din("ones_c", [128, 1], bf16)
    ones_fc = din("ones_fc", [128, 1])
    eps_in = din("eps_c", [128, 1])
    eps1_in = din("eps1_c", [1, 1])

    outT = nc.dram_tensor("outT", [DIM, QS], f32, kind="ExternalOutput").ap()

    import os as _os
    _sim = _os.environ.get("KSIM", "0") == "1"
    _dbg = _os.environ.get("KDBG", "0") == "1"
    # cat: per-core partial numerators+denominators, query-block-major so a
    # ReduceScatter hands core c exactly its 64-query slice.
    # rows 0..1024: numerator (h*128+d), rows 1024..1032: denominator per head
    cat = nc.dram_tensor("cat", [N_CORES, DIM + HEADS, QS], f32)
    catrs = nc.dram_tensor("catrs", [DIM + HEADS, QS], f32)
    if _dbg:
        qdbg = nc.dram_tensor("qdbg", [128, 8, SQ], bf16, kind="ExternalOutput").ap()
        krdbg = nc.dram_tensor("krdbg", [128, 8, skc], bf16, kind="ExternalOutput").ap()
        rsbdbg = nc.dram_tensor("rsbdbg", [128, tt], f32, kind="ExternalOutput").ap()
        catdbg = nc.dram_tensor("catdbg", [N_CORES, DIM + HEADS, QS], f32, kind="ExternalOutput").ap()
        nscdbg = nc.dram_tensor("nscdbg", [128, 8, QS], f32, kind="ExternalOutput").ap()

    with tile.TileContext(nc) as tc:
        ctx_pools = []

        def pool(name, bufs, space=None):
            kw = dict(name=name, bufs=bufs)
            if space:
                kw["space"] = space
            p = tc.tile_pool(**kw)
            ctx_pools.append(p)
            return p.__enter__()

        # pool creation order fixes SBUF address ranges: the weight/chunk pools
        # must NOT overlap the Q-phase pools, or their early DMAs would wait on
        # the Q pipeline's last reads (observed: wk arriving at ~57us)
        consts = pool("consts", 1)
        resid = pool("resid", 1)
        wpool = pool("wpool", 1)
        kpool = pool("kpool", 2)
        kpool1 = pool("kpool1", 1)
        pp = pool("pp", 3, space="PSUM")
        pp_acc = pool("pp_acc", 3, space="PSUM")
        pp_den = pool("pp_den", 1, space="PSUM")
        pp_sq = pool("pp_sq", 1, space="PSUM")

        # ---- constants / resident tensors (xT + first Q weight slices first:
        # they gate the very first matmuls) ----
        qpf_cm = tc.tile_pool(name="qpf", bufs=2)
        qpf = qpf_cm.__enter__()
        wqpf_cm = tc.tile_pool(name="wqpf", bufs=3)
        wqpf = wqpf_cm.__enter__()
        xt_s = consts.tile([128, 8, SQ], bf16)
        nc.sync.dma_start(xt_s[:], xT.rearrange("(i p) q -> p i q", p=128))
        wq_tiles = {}
        rope_q_tiles = {}
        for o in (0, 1, 2):
            w_t = wqpf.tile([128, 8, 128], bf16, tag="wq_o")
            (nc.sync if o % 2 == 0 else nc.scalar).dma_start(w_t[:], wq[:, :, o, :])
            wq_tiles[o] = w_t
        pt_s = consts.tile([128, 128], bf16)
        nc.sync.dma_start(pt_s[:], pmat)
        wk_s = wpool.tile([128, 8, 8, 128], bf16)
        wv_s = wpool.tile([128, 8, DIM], bf16)
        ones_s = consts.tile([128, 1], bf16)
        nc.sync.dma_start(ones_s[:], ones_c)
        ones_fs = consts.tile([128, 1], f32)
        nc.sync.dma_start(ones_fs[:], ones_fc)
        bq_s = consts.tile([128, 8], f32)
        nc.sync.dma_start(bq_s[:], bq_t)
        eps1_s = consts.tile([1, 1], f32)
        nc.sync.dma_start(eps1_s[:], eps1_in)
        atl_d = consts.tile([1, 1], f32)
        nc.sync.dma_start(wk_s[:], wk)

        def load_chunk(ct0, ntt):
            cw = ntt * 128
            c0 = ct0 * 128
            memt = kpool.tile([128, 8, cw], bf16, tag="memt")
            nc.sync.dma_start(
                memt[:], memT[:, c0:c0 + cw].rearrange("(i p) t -> p i t", p=128))
            ctk_t = kpool.tile([128, cw], bf16, tag="ctk")
            nc.sync.dma_start(ctk_t[:], ctk[:, c0:c0 + cw])
            stk_t = kpool.tile([128, cw], bf16, tag="stk")
            nc.sync.dma_start(stk_t[:], stk[:, c0:c0 + cw])
            return memt, ctk_t, stk_t

        k0_tiles = load_chunk(0, min(CHUNK_TILES, tt))
        nc.sync.dma_start(wv_s[:], wv)

        qT = resid.tile([128, 8, SQ], bf16)      # rope'd+normalized Q (pre-scaled)
        kr = resid.tile([128, 8, skc], bf16)     # rope'd UN-normalized K
        v_sb = resid.tile([128, tt, DIM], bf16)  # V, token-major
        rsb_all = resid.tile([128, tt], f32)     # 1/rms per key, tile-column layout
        mb_s = resid.tile([128, tt], f32)
        bk_s = consts.tile([128, 8], f32)
        eps_s = consts.tile([128, 1], f32)
        bo_s = consts.tile([128, 8], f32)
        nc.sync.dma_start(bo_s[:], bo_t)

        # =========== Q projection (rope/normalize deferred into phase 1) ======
        qpool_cm = tc.tile_pool(name="qpool", bufs=1)
        qpool = qpool_cm.__enter__()
        ybq = qpool.tile([128, 8, SQ], bf16, tag="ybq")
        ysq_q = qpool.tile([128, 8, SQ], bf16, tag="ysq")
        for o in range(8):
            if o in wq_tiles:
                wq_o = wq_tiles.pop(o)
            else:
                wq_o = wqpf.tile([128, 8, 128], bf16, tag="wq_o")
                (nc.sync if o % 2 == 0 else nc.scalar).dma_start(
                    wq_o[:], wq[:, :, o, :])
            ps_q = pp.tile([128, SQ], f32, tag="ps")
            for i in range(8):
                nc.tensor.matmul(ps_q[:], wq_o[:, i, :], xt_s[:, i, :],
                                 start=(i == 0), stop=(i == 7))
            nc.scalar.activation(ybq[:, o, :], ps_q[:], AF.Identity,
                                 bias=bq_s[:, o:o + 1])
            nc.vector.tensor_mul(ysq_q[:, o, :], ybq[:, o, :], ybq[:, o, :])
        # deferred const loads (off the critical first DMAs)
        nc.sync.dma_start(bk_s[:], bk_t)
        nc.sync.dma_start(eps_s[:], eps_in)
        nc.sync.dma_start(mb_s[:], mbias)

        def q_swap_work(o):
            # one Q rope step; sprinkled through the last chunk's K projection
            # so the dense PE work hides the ACT/DVE consumers
            if o in rope_q_tiles:
                ctq_o, stq_o = rope_q_tiles.pop(o)
            else:
                ctq_o = qpf.tile([128, SQ], bf16, tag="ctq_o")
                nc.scalar.dma_start(ctq_o[:], ctq[:, o, :])
                stq_o = qpf.tile([128, SQ], bf16, tag="stq_o")
                nc.scalar.dma_start(stq_o[:], stq[:, o, :])
            ps_sw = pp_acc.tile([128, SQ], f32, tag="acc")
            nc.tensor.matmul(ps_sw[:], pt_s[:], ybq[:, o, :])
            swb = qpool.tile([128, SQ], bf16, tag=f"swb_{o % 2}")
            nc.scalar.activation(swb[:], ps_sw[:], AF.Identity)
            t1 = qpool.tile([128, SQ], bf16, tag="t1")
            nc.vector.tensor_mul(t1[:], ybq[:, o, :], ctq_o[:])
            t2 = qpool.tile([128, SQ], bf16, tag="t2")
            nc.vector.tensor_mul(t2[:], swb[:], stq_o[:])
            nc.vector.tensor_add(qT[:, o, :], t1[:], t2[:])

        def q_finish():
            # Q sum-of-squares + normalization chain
            ps_sq_q = pp_den.tile([1, SQ], f32, tag="den")
            for o in range(8):
                nc.tensor.matmul(ps_sq_q[:], ones_s[:], ysq_q[:, o, :],
                                 start=(o == 0), stop=(o == 7))
            sq_q = qpool.tile([1, SQ], f32, tag="sqr")
            nc.scalar.activation(sq_q[:], ps_sq_q[:], AF.Sqrt,
                                 bias=eps1_s[:], scale=1.0 / DIM)
            rs_q = qpool.tile([1, SQ], f32, tag="rs")
            nc.vector.reciprocal(rs_q[:], sq_q[:])
            rsb_q = qpool.tile([128, SQ], f32, tag="rsb")
            nc.gpsimd.partition_broadcast(rsb_q[:], rs_q[:])
            for o in range(8):
                nc.vector.tensor_mul(qT[:, o, :], qT[:, o, :], rsb_q[:])

        # =========== phase 1: K rope + V for all key tiles ===========

        def make_sumsq(ct0, ntt, ysq_all):
            # deferred: emitted after the NEXT chunk's K projection so the PE
            # never waits on this chunk's ysq (DVE) completion
            def emit():
                pst = pp_sq.tile([128, ntt], f32, tag="pst")
                for ti in range(ntt):
                    for o in range(8):
                        nc.tensor.matmul(
                            pst[:, ti:ti + 1],
                            ysq_all[:, o, ti * 128:(ti + 1) * 128], ones_s[:],
                            start=(o == 0), stop=(o == 7))
                sq_t = kpool1.tile([128, ntt], f32, tag="sqt")
                nc.scalar.activation(sq_t[:], pst[:], AF.Sqrt,
                                     bias=eps_s[:], scale=1.0 / DIM)
                nc.vector.reciprocal(rsb_all[:, ct0:ct0 + ntt], sq_t[:])
            return emit

        pend_sumsq = None
        for ct0 in range(0, tt, CHUNK_TILES):
            last = ct0 + CHUNK_TILES >= tt
            ntt = min(CHUNK_TILES, tt - ct0)
            cw = ntt * 128
            c0 = ct0 * 128
            if ct0 == 0:
                memt, ctk_t, stk_t = k0_tiles
            else:
                memt, ctk_t, stk_t = load_chunk(ct0, ntt)

            yk = kpool.tile([128, 8, cw], bf16, tag="yk")
            ysq_all = kpool.tile([128, 8, cw], bf16, tag="ysq")
            t1_tiles = {}

            def rope_finish(op, ps_sw, c0=c0, cw=cw, stk_t=stk_t,
                            t1_tiles=t1_tiles):
                # kr[op] = yk[op]*cos + (P yk[op])*sin   (1/rms deferred to Exp)
                swb = kpool1.tile([128, cw], bf16, tag=f"swb_{op % 2}")
                nc.scalar.activation(swb[:], ps_sw[:], AF.Identity)
                t2 = kpool1.tile([128, cw], bf16, tag="t2")
                nc.vector.tensor_mul(t2[:], swb[:], stk_t[:])
                nc.vector.tensor_add(kr[:, op, c0:c0 + cw],
                                     t1_tiles.pop(op)[:], t2[:])

            sw_prev = None
            for o in range(8):
                ps_y = pp.tile([128, cw], f32, tag="ps")
                for i in range(8):
                    nc.tensor.matmul(ps_y[:], wk_s[:, i, o, :], memt[:, i, :],
                                     start=(i == 0), stop=(i == 7))
                if sw_prev is not None:
                    # swap matmul for o-1 (after o's projection, so the PE
                    # never waits on yk[o-1]'s activation copy)
                    ps_sw = pp_acc.tile([128, cw], f32, tag="acc")
                    nc.tensor.matmul(ps_sw[:], pt_s[:], yk[:, sw_prev, :])
                    rope_pend = (sw_prev, ps_sw)
                else:
                    rope_pend = None
                nc.scalar.activation(yk[:, o, :], ps_y[:], AF.Identity,
                                     bias=bk_s[:, o:o + 1])
                nc.vector.tensor_mul(ysq_all[:, o, :], yk[:, o, :], yk[:, o, :])
                t1 = kpool1.tile([128, cw], bf16, tag=f"t1_{o % 2}")
                nc.vector.tensor_mul(t1[:], yk[:, o, :], ctk_t[:])
                t1_tiles[o] = t1
                if rope_pend is not None:
                    rope_finish(*rope_pend)
                if last:
                    q_swap_work(o)
                sw_prev = o
            ps_sw = pp_acc.tile([128, cw], f32, tag="acc")
            nc.tensor.matmul(ps_sw[:], pt_s[:], yk[:, 7, :])
            rope_finish(7, ps_sw)
            if last:
                q_finish()

            if pend_sumsq is not None:
                pend_sumsq()

            # V projection (token-major output)
            for ti in range(ntt):
                for oh in range(2):
                    ps_v = pp.tile([128, 512], f32, tag="ps")
                    for i in range(8):
                        nc.tensor.matmul(
                            ps_v[:], memt[:, i, ti * 128:(ti + 1) * 128],
                            wv_s[:, i, oh * 512:(oh + 1) * 512],
                            start=(i == 0), stop=(i == 7))
                    nc.scalar.activation(
                        v_sb[:, ct0 + ti, oh * 512:(oh + 1) * 512], ps_v[:],
                        AF.Identity)
                if last and ti == 1:
                    # last chunk: emit its own sum-sq early so the final Sqrt
                    # (and the Exp-table warm behind it) clears the ACT queue
                    # well before phase 2's first exp
                    make_sumsq(ct0, ntt, ysq_all)()
                    nc.scalar.activation(atl_d[:], eps1_s[:], AF.Exp)

            if not last:
                pend_sumsq = make_sumsq(ct0, ntt, ysq_all)
        qpool_cm.__exit__(None, None, None)
        wqpf_cm.__exit__(None, None, None)
        qpf_cm.__exit__(None, None, None)

        # =========== phase 2: per-head attention, PSUM accumulation ===========
        wopool = pool("wopool", 1)
        wo_s = wopool.tile([128, 8, 8, 128], bf16)
        nc.sync.dma_start(wo_s[:], wo)
        ppool_cm = tc.tile_pool(name="ppool", bufs=4)
        ppool = ppool_cm.__enter__()
        p2_cm = tc.tile_pool(name="p2", bufs=2)
        p2 = p2_cm.__enter__()
        p1_cm = tc.tile_pool(name="p1", bufs=1)
        p1 = p1_cm.__enter__()
        LOOK = 3
        den_split = tt >= 4
        for h in range(8):
            ps_n = pp_acc.tile([128, SQ], f32, tag="acc")
            den_d = pp_den.tile([1, SQ], f32, tag="den")
            if den_split:
                dacc = p1.tile([128, SQ], f32, tag="dacc")
            else:
                dacc = None
            pts = {}

            def consume(t, h=h, ps_n=ps_n, den_d=den_d, dacc=dacc, pts=pts):
                nc.tensor.matmul(ps_n[:], v_sb[:, t, h * 128:(h + 1) * 128],
                                 pts[t][:], start=(t == 0), stop=(t == tt - 1))
                # denominator: even tiles on the PE (PSUM row), odd tiles on
                # the otherwise-idle DVE; combined by a final f32 matmul.
                # This drops phase 2 from PE-paced to ACT(exp)-paced.
                if den_split and t % 2 == 1:
                    if t == 1:
                        nc.vector.tensor_copy(dacc[:], pts[t][:])
                    else:
                        nc.vector.tensor_add(dacc[:], dacc[:], pts[t][:])
                else:
                    nc.tensor.matmul(den_d[:], ones_s[:], pts[t][:],
                                     start=(t == 0),
                                     stop=(not den_split and t == tt - 1))
                del pts[t]

            for t in range(tt):
                ps_s = pp.tile([128, SQ], f32, tag="ps")
                nc.tensor.matmul(ps_s[:], kr[:, h, t * 128:(t + 1) * 128],
                                 qT[:, h, :])
                pt = ppool.tile([128, SQ], bf16, tag="pt")
                nc.scalar.activation(pt[:], ps_s[:], AF.Exp,
                                     bias=mb_s[:, t:t + 1],
                                     scale=rsb_all[:, t:t + 1])
                pts[t] = pt
                if t >= LOOK:
                    consume(t - LOOK)
            for t in range(max(0, tt - LOOK), tt):
                consume(t)
            if den_split:
                nc.tensor.matmul(den_d[:], ones_fs[:], dacc[:],
                                 start=False, stop=True)

            dsb = p2.tile([1, SQ], f32, tag="dsb")
            nc.vector.tensor_copy(dsb[:], den_d[:])
            nsb = p2.tile([128, SQ], f32, tag="nsb")
            nc.vector.tensor_copy(nsb[:], ps_n[:])
            nc.scalar.dma_start(
                cat[:, h * 128:(h + 1) * 128, :].rearrange("b p q -> p b q"),
                nsb[:])
            nc.gpsimd.dma_start(
                cat[:, DIM + h:DIM + h + 1, :].rearrange("b o q -> o b q"),
                dsb[:])
        p1_cm.__exit__(None, None, None)
        p2_cm.__exit__(None, None, None)
        ppool_cm.__exit__(None, None, None)

        # =========== reduce-scatter across cores (query axis) ===========
        tail = pool("tail", 1)
        if _dbg:
            nc.sync.dma_start(qdbg, qT[:])
            nc.sync.dma_start(krdbg, kr[:])
            nc.sync.dma_start(rsbdbg, rsb_all[:])
            nc.sync.dma_start(catdbg, cat[:, :, :])
        if _sim:
            nc.gpsimd.dma_start(catrs[:], cat[0, :, :])
        else:
            nc.gpsimd.collective_compute(
                "ReduceScatter", mybir.AluOpType.add,
                replica_groups=[list(range(N_CORES))],
                ins=[cat[:]], outs=[catrs[:]])

        # =========== per-core tail: normalize + output projection ===========
        dden = tail.tile([1, 8, QS], f32)
        nc.scalar.dma_start(dden[:], catrs[DIM:DIM + HEADS, :])
        nred = tail.tile([128, 8, QS], f32)
        nc.scalar.dma_start(
            nred[:], catrs[0:DIM, :].rearrange("(h p) q -> p h q", p=128))
        ddb = tail.tile([128, 8, QS], f32)
        nc.gpsimd.partition_broadcast(ddb[:], dden[:])
        rdb = tail.tile([128, 8, QS], f32)
        nc.vector.reciprocal(rdb[:], ddb[:])
        nsc = tail.tile([128, 8, QS], bf16)
        nc.vector.tensor_mul(nsc[:], nred[:], rdb[:])
        out_sb = tail.tile([128, 8, QS], f32)
        for e in range(8):
            ps_o = pp.tile([128, QS], f32, tag="ps")
            for o in range(8):
                nc.tensor.matmul(ps_o[:], wo_s[:, o, e, :], nsc[:, o, :],
                                 start=(o == 0), stop=(o == 7))
            nc.scalar.activation(out_sb[:, e, :], ps_o[:], AF.Identity,
                                 bias=bo_s[:, e:e + 1])
        if _dbg:
            nscf = tail.tile([128, 8, QS], f32)
            nc.vector.tensor_copy(nscf[:], nsc[:])
            nc.sync.dma_start(nscdbg, nscf[:])
        nc.sync.dma_start(outT.rearrange("(e p) q -> p e q", p=128), out_sb[:])

        for p in reversed(ctx_pools):
            p.__exit__(None, None, None)

    nc.compile()
    _cache[key] = nc
    return nc


def _prep(x, mem, mask, cos_q, sin_q, cos_k, sin_k,
          Wq, bq, Wk, bk, Wv, bv, Wo, bo, gq, gk):
    import ml_dtypes
    f = np.float32
    bf = ml_dtypes.bfloat16
    x = np.asarray(x, f).reshape(SQ, DIM)
    mem = np.asarray(mem, f)
    mem = mem.reshape(-1, DIM)
    sk = mem.shape[0]
    mask = np.asarray(mask).reshape(sk)
    cos_q = np.asarray(cos_q, f)
    sin_q = np.asarray(sin_q, f)
    cos_k = np.asarray(cos_k, f)
    sin_k = np.asarray(sin_k, f)
    Wq, Wk, Wv, Wo = (np.asarray(w, f) for w in (Wq, Wk, Wv, Wo))
    bq, bk, bv, bo, gq, gk = (np.asarray(v, f) for v in (bq, bk, bv, bo, gq, gk))

    assert np.all(bv == 0.0), "nonzero bv not supported by this build"
    if not np.allclose(gk, 1.0):
        gkp = gk.reshape(-1, 2)
        assert np.allclose(gkp[:, 0], gkp[:, 1]), "unsupported non-pairwise gk"

    def tile_w(WT):  # [1024,1024] (in,out of W.T) -> [p, i, o, m]
        return np.ascontiguousarray(
            WT.reshape(8, 128, 8, 128).transpose(1, 0, 2, 3)).astype(bf)

    ii = np.arange(128)
    jj = ii // 2
    partner = ii ^ 1

    # fold gq (+pairwise gk) and the 1/sqrt(HD) score scale into the q rope
    # tables; sin pairs with partner's gq
    gq_t = (gq * gk).reshape(8, 128) * SCALE
    gq_sin = (gq.reshape(8, 128)[:, partner] * gk.reshape(8, 128)) * SCALE
    cq = cos_q[:, jj].T                # [128, SQ]
    sq = sin_q[:, jj].T
    ctq = np.ascontiguousarray(
        (cq[None, :, :] * gq_t[:, :, None]).transpose(1, 0, 2)).astype(bf)
    stq = np.ascontiguousarray(
        (sq[None, :, :] * gq_sin[:, :, None]).transpose(1, 0, 2)).astype(bf)

    PT = np.zeros((128, 128), f)
    even = ii[ii % 2 == 0]
    PT[even + 1, even] = -1.0
    PT[even, even + 1] = 1.0

    shared = {
        "xT": np.ascontiguousarray(x.T).astype(bf),
        "wq": tile_w(Wq.T), "wk": tile_w(Wk.T), "wo": tile_w(Wo.T),
        "wv": np.ascontiguousarray(
            Wv.T.reshape(8, 128, DIM).transpose(1, 0, 2)).astype(bf),
        "ctq": ctq, "stq": stq,
        "bq_t": np.ascontiguousarray(bq.reshape(8, 128).T),
        "bk_t": np.ascontiguousarray(bk.reshape(8, 128).T),
        "bo_t": np.ascontiguousarray(bo.reshape(8, 128).T),
        "pmat": PT.astype(bf),
        "ones_c": np.ones((128, 1), bf),
        "ones_fc": np.ones((128, 1), f),
        "eps_c": np.full((128, 1), EPS, f),
        "eps1_c": np.full((1, 1), EPS, f),
    }

    # ---- mask compaction: keep only unmasked keys, pad to 8*tt*128 ----
    idx = np.flatnonzero(mask)
    m = idx.size
    tt = max(1, -(-m // (N_CORES * 128)))
    skc = tt * 128
    total = N_CORES * skc

    memc = np.zeros((total, DIM), f)
    memc[:m] = mem[idx]
    ck = np.zeros((total, HD // 2), f)
    sk_ = np.zeros((total, HD // 2), f)
    ck[:m] = cos_k[idx]
    sk_[:m] = sin_k[idx]
    mb_full = np.full(total, NEG, f)
    mb_full[:m] = 0.0

    in_maps = []
    for c in range(N_CORES):
        s = slice(c * skc, (c + 1) * skc)
        mmap = dict(shared)
        mmap["memT"] = np.ascontiguousarray(memc[s].T).astype(bf)
        mmap["ctk"] = np.ascontiguousarray(ck[s][:, jj].T).astype(bf)
        mmap["stk"] = np.ascontiguousarray(sk_[s][:, jj].T).astype(bf)
        mmap["mbias"] = np.ascontiguousarray(mb_full[s].reshape(tt, 128).T)
        in_maps.append(mmap)
    return in_maps, tt


def kernel(**inputs):
    global _last_tt
    from concourse.bass_utils import run_bass_kernel_spmd
    in_maps, tt = _prep(**inputs)
    _last_tt = tt
    nc = _build(tt)
    res = run_bass_kernel_spmd(nc, in_maps, list(range(N_CORES)))
    parts = [res.results[c]["outT"].T for c in range(N_CORES)]
    out = np.concatenate(parts, axis=0)
    return out[None].astype(np.float32)


# revision 66
# speedup vs baseline: 1.2741x; 1.0538x over previous
"""Trainium2 Bass kernel for nn_MemoryRetriever (cross-attention memory retriever).

Strategy (v2):
- Host-side mask compaction: only unmasked memory tokens (~50%) are sharded
  across the 8 cores; padding keys are killed with a -1e30 exp bias.  Exact
  same math (masked keys contribute exactly zero), ~2x less device work.
- Phase 1 per core: K projection + 3D-RoPE + transposed sum-of-squares for
  RMSNorm, and V projection, for the core's key shard; K (roped,
  un-normalized) and V stay resident in SBUF.  The 1/rms factor is NOT
  multiplied into K; it is folded into the Exp activation's per-partition
  scale AP later (keys sit on partitions in the score tiles).
- Phase 2 per head: scores = K_raw^T Q (Q pre-scaled by 1/sqrt(128)), Exp
  with scale=rsqrt-AP and bias=mask-AP, then attn@V and the softmax
  denominator both accumulate in PSUM across all key tiles (start/stop
  accumulation groups) - no vector-engine accumulators.  Results DMA
  straight from PSUM to DRAM in a query-block-major layout.
- One ReduceScatter (query axis) replaces the baseline AllReduce: each core
  receives exactly its 64-query slice of the summed numerators/denominators,
  normalizes, and output-projects it.  Host concatenates the 8 slices.

All matmul operands bf16 (fp32 PSUM accumulation); softmax/normalization
arithmetic fp32.
"""

import math
import sys

import numpy as np

sys.path.insert(0, "/opt/trn_rl_repo")

DIM = 1024
HEADS = 8
HD = 128
SQ = 512
N_CORES = 8
QS = SQ // N_CORES   # 64 queries per core in the tail
EPS = 1e-6
SCALE = 1.0 / math.sqrt(128.0)
NEG = -1.0e30
CHUNK_TILES = 4      # key tiles per phase-1 chunk

_cache = {}
_last_tt = None


def _build(tt=16):
    """Build + compile the per-core program for a shard of `tt` key tiles."""
    key = ("nc", tt)
    if key in _cache:
        return _cache[key]

    import concourse.bass as bass  # noqa: F401
    import concourse.tile as tile
    from concourse import mybir, bacc

    f32 = mybir.dt.float32
    bf16 = mybir.dt.bfloat16
    AF = mybir.ActivationFunctionType

    skc = tt * 128

    nc = bacc.Bacc("TRN2", target_bir_lowering=False, debug=False,
                   num_devices=N_CORES)

    def din(name, shape, dt=f32):
        return nc.dram_tensor(name, list(shape), dt, kind="ExternalInput").ap()

    # per-core sharded inputs (compacted keys)
    memT = din("memT", [DIM, skc], bf16)    # mem shard, feature-major
    ctk = din("ctk", [HD, skc], bf16)       # K rope cos table (in-head d major)
    stk = din("stk", [HD, skc], bf16)
    mbias = din("mbias", [128, tt])         # exp bias (0 valid / -1e30 pad)
    # shared inputs
    xT = din("xT", [DIM, SQ], bf16)
    wq = din("wq", [8, 128, 8, 128], bf16)  # [o,p,i,m]: o-major so per-o DMA slices are contiguous
    wk = din("wk", [128, 8, 8, 128], bf16)
    wo = din("wo", [128, 8, 8, 128], bf16)  # [p,o,e,m] = Wo.T[o*128+p, e*128+m]
    wv = din("wv", [128, 8, DIM], bf16)     # [p,i,o] = Wv.T[i*128+p, o]
    ctq = din("ctq", [128, 8, SQ], bf16)    # q rope cos (gq*gk*SCALE folded)
    stq = din("stq", [128, 8, SQ], bf16)
    bq_t = din("bq_t", [128, 8])
    bk_t = din("bk_t", [128, 8])
    bo_t = din("bo_t", [128, 8])
    pmat = din("pmat", [128, 128], bf16)    # P.T for rope pair swap (+-1)
    ones_c = din("ones_c", [128, 1], bf16)
    ones_fc = din("ones_fc", [128, 1])
    eps_in = din("eps_c", [128, 1])
    eps1_in = din("eps1_c", [1, 1])

    outT = nc.dram_tensor("outT", [DIM, QS], f32, kind="ExternalOutput").ap()

    import os as _os
    _sim = _os.environ.get("KSIM", "0") == "1"
    _dbg = _os.environ.get("KDBG", "0") == "1"
    # cat: per-core partial numerators+denominators, query-block-major so a
    # ReduceScatter hands core c exactly its 64-query slice.
    # rows 0..1024: numerator (h*128+d), rows 1024..1032: denominator per head
    HROWS = DIM // 2 + HEADS // 2   # 516: 4 heads' numerators + denominators
    cat_a = nc.dram_tensor("cat_a", [N_CORES, HROWS, QS], f32)
    cat_b = nc.dram_tensor("cat_b", [N_CORES, HROWS, QS], f32)
    catrs_a = nc.dram_tensor("catrs_a", [HROWS, QS], f32)
    catrs_b = nc.dram_tensor("catrs_b", [HROWS, QS], f32)
    if _dbg:
        qdbg = nc.dram_tensor("qdbg", [128, 8, SQ], bf16, kind="ExternalOutput").ap()
        krdbg = nc.dram_tensor("krdbg", [128, 8, skc], bf16, kind="ExternalOutput").ap()
        rsbdbg = nc.dram_tensor("rsbdbg", [128, tt], f32, kind="ExternalOutput").ap()
        catdbg = nc.dram_tensor("catdbg", [N_CORES, DIM + HEADS, QS], f32, kind="ExternalOutput").ap()
        nscdbg = nc.dram_tensor("nscdbg", [128, 8, QS], f32, kind="ExternalOutput").ap()

    with tile.TileContext(nc) as tc:
        ctx_pools = []

        def pool(name, bufs, space=None):
            kw = dict(name=name, bufs=bufs)
            if space:
                kw["space"] = space
            p = tc.tile_pool(**kw)
            ctx_pools.append(p)
            return p.__enter__()

        # pool creation order fixes SBUF address ranges: the weight/chunk pools
        # must NOT overlap the Q-phase pools, or their early DMAs would wait on
        # the Q pipeline's last reads (observed: wk arriving at ~57us)
        consts = pool("consts", 1)
        resid = pool("resid", 1)
        wpool = pool("wpool", 1)
        kpool = pool("kpool", 2)
        kpool1 = pool("kpool1", 1)
        pp = pool("pp", 3, space="PSUM")
        pp_acc = pool("pp_acc", 3, space="PSUM")
        pp_den = pool("pp_den", 1, space="PSUM")
        pp_sq = pool("pp_sq", 1, space="PSUM")

        # ---- constants / resident tensors (xT + first Q weight slices first:
        # they gate the very first matmuls) ----
        qpf_cm = tc.tile_pool(name="qpf", bufs=2)
        qpf = qpf_cm.__enter__()
        wqpf_cm = tc.tile_pool(name="wqpf", bufs=4)
        wqpf = wqpf_cm.__enter__()
        qpool_cm = tc.tile_pool(name="qpool", bufs=1)
        qpool = qpool_cm.__enter__()
        wq_tiles = {}
        rope_q_tiles = {}
        w_t0 = wqpf.tile([128, 8, 128], bf16, tag="wq_o")
        nc.sync.dma_start(w_t0[:], wq[0, :, :, :])
        wq_tiles[0] = w_t0
        xt_s = qpool.tile([128, 8, SQ], bf16, tag="xt")
        nc.sync.dma_start(xt_s[:], xT.rearrange("(i p) q -> p i q", p=128))
        for o in (1, 2, 3):
            w_t = wqpf.tile([128, 8, 128], bf16, tag="wq_o")
            nc.sync.dma_start(w_t[:], wq[o, :, :, :])
            wq_tiles[o] = w_t
        def load_chunk(ct0, ntt):
            cw = ntt * 128
            c0 = ct0 * 128
            memt = kpool.tile([128, 8, cw], bf16, tag="memt")
            nc.sync.dma_start(
                memt[:], memT[:, c0:c0 + cw].rearrange("(i p) t -> p i t", p=128))
            ctk_t = kpool.tile([128, cw], bf16, tag="ctk")
            nc.sync.dma_start(ctk_t[:], ctk[:, c0:c0 + cw])
            stk_t = kpool.tile([128, cw], bf16, tag="stk")
            nc.sync.dma_start(stk_t[:], stk[:, c0:c0 + cw])
            return memt, ctk_t, stk_t

        bq_s = consts.tile([128, 8], f32)
        nc.scalar.dma_start(bq_s[:], bq_t)
        wk_s = wpool.tile([128, 8, 8, 128], bf16)
        wv_s = wpool.tile([128, 8, DIM], bf16)
        pt_s = consts.tile([128, 128], bf16)
        nc.scalar.dma_start(pt_s[:], pmat)
        ones_s = consts.tile([128, 1], bf16)
        nc.scalar.dma_start(ones_s[:], ones_c)
        ones_fs = consts.tile([128, 1], f32)
        nc.scalar.dma_start(ones_fs[:], ones_fc)
        eps1_s = consts.tile([1, 1], f32)
        nc.scalar.dma_start(eps1_s[:], eps1_in)
        atl_d = consts.tile([1, 1], f32)

        qT = resid.tile([128, 8, SQ], bf16)      # rope'd+normalized Q (pre-scaled)
        kr = resid.tile([128, 8, skc], bf16)     # rope'd UN-normalized K
        v_sb = resid.tile([128, tt, DIM], bf16)  # V, token-major
        rsb_all = resid.tile([128, tt], f32)     # 1/rms per key, tile-column layout
        mb_s = resid.tile([128, tt], f32)
        bk_s = consts.tile([128, 8], f32)
        eps_s = consts.tile([128, 1], f32)
        bo_s = consts.tile([128, 8], f32)
        nc.sync.dma_start(bo_s[:], bo_t)

        # =========== Q projection (rope/normalize deferred into phase 1) ======
        ybq = qpool.tile([128, 8, SQ], bf16, tag="ybq")
        ysq_q = qpool.tile([128, 8, SQ], bf16, tag="ysq")
        for o in range(8):
            if o in wq_tiles:
                wq_o = wq_tiles.pop(o)
            else:
                wq_o = wqpf.tile([128, 8, 128], bf16, tag="wq_o")
                nc.sync.dma_start(wq_o[:], wq[o, :, :, :])
            ps_q = pp.tile([128, SQ], f32, tag="ps")
            for i in range(8):
                nc.tensor.matmul(ps_q[:], wq_o[:, i, :], xt_s[:, i, :],
                                 start=(i == 0), stop=(i == 7))
            nc.scalar.activation(ybq[:, o, :], ps_q[:], AF.Identity,
                                 bias=bq_s[:, o:o + 1])
            nc.vector.tensor_mul(ysq_q[:, o, :], ybq[:, o, :], ybq[:, o, :])
        # weight + chunk-0 loads: their transfers slot into the serial DMA
        # pipe right behind the ring-dispatched wq slices, arriving just in
        # time for the K projection
        nc.sync.dma_start(wk_s[:], wk)
        k0_tiles = load_chunk(0, min(CHUNK_TILES, tt))
        nc.sync.dma_start(bk_s[:], bk_t)
        nc.sync.dma_start(eps_s[:], eps_in)
        nc.sync.dma_start(mb_s[:], mbias)
        nc.sync.dma_start(wv_s[:], wv)

        def q_swap_work(o):
            # one Q rope step; sprinkled through the last chunk's K projection
            # so the dense PE work hides the ACT/DVE consumers
            if o in rope_q_tiles:
                ctq_o, stq_o = rope_q_tiles.pop(o)
            else:
                ctq_o = qpf.tile([128, SQ], bf16, tag="ctq_o")
                nc.scalar.dma_start(ctq_o[:], ctq[:, o, :])
                stq_o = qpf.tile([128, SQ], bf16, tag="stq_o")
                nc.scalar.dma_start(stq_o[:], stq[:, o, :])
            ps_sw = pp_acc.tile([128, SQ], f32, tag="acc")
            nc.tensor.matmul(ps_sw[:], pt_s[:], ybq[:, o, :])
            swb = qpool.tile([128, SQ], bf16, tag=f"swb_{o % 2}")
            nc.scalar.activation(swb[:], ps_sw[:], AF.Identity)
            t1 = qpool.tile([128, SQ], bf16, tag="t1")
            nc.vector.tensor_mul(t1[:], ybq[:, o, :], ctq_o[:])
            t2 = qpool.tile([128, SQ], bf16, tag="t2")
            nc.vector.tensor_mul(t2[:], swb[:], stq_o[:])
            nc.vector.tensor_add(qT[:, o, :], t1[:], t2[:])

        def q_finish():
            # Q sum-of-squares + normalization chain
            ps_sq_q = pp_den.tile([1, SQ], f32, tag="den")
            for o in range(8):
                nc.tensor.matmul(ps_sq_q[:], ones_s[:], ysq_q[:, o, :],
                                 start=(o == 0), stop=(o == 7))
            sq_q = qpool.tile([1, SQ], f32, tag="sqr")
            nc.scalar.activation(sq_q[:], ps_sq_q[:], AF.Sqrt,
                                 bias=eps1_s[:], scale=1.0 / DIM)
            nc.vector.reciprocal(sq_q[:], sq_q[:])
            rsb_q = qpool.tile([128, SQ], f32, tag="rsb")
            nc.gpsimd.partition_broadcast(rsb_q[:], sq_q[:])
            for o in range(8):
                nc.vector.tensor_mul(qT[:, o, :], qT[:, o, :], rsb_q[:])

        # =========== phase 1: K rope + V for all key tiles ===========

        def make_sumsq(ct0, ntt, ysq_all):
            # deferred: emitted after the NEXT chunk's K projection so the PE
            # never waits on this chunk's ysq (DVE) completion
            def emit():
                pst = pp_sq.tile([128, ntt], f32, tag="pst")
                for ti in range(ntt):
                    for o in range(8):
                        nc.tensor.matmul(
                            pst[:, ti:ti + 1],
                            ysq_all[:, o, ti * 128:(ti + 1) * 128], ones_s[:],
                            start=(o == 0), stop=(o == 7))
                sq_t = kpool1.tile([128, ntt], f32, tag="sqt")
                nc.scalar.activation(sq_t[:], pst[:], AF.Sqrt,
                                     bias=eps_s[:], scale=1.0 / DIM)
                nc.vector.reciprocal(rsb_all[:, ct0:ct0 + ntt], sq_t[:])
            return emit

        pend_sumsq = None
        for ct0 in range(0, tt, CHUNK_TILES):
            last = ct0 + CHUNK_TILES >= tt
            ntt = min(CHUNK_TILES, tt - ct0)
            cw = ntt * 128
            c0 = ct0 * 128
            if ct0 == 0:
                memt, ctk_t, stk_t = k0_tiles
            else:
                memt, ctk_t, stk_t = load_chunk(ct0, ntt)

            yk = kpool.tile([128, 8, cw], bf16, tag="yk")
            ysq_all = kpool.tile([128, 8, cw], bf16, tag="ysq")
            t1_tiles = {}

            def rope_finish(op, ps_sw, c0=c0, cw=cw, stk_t=stk_t,
                            t1_tiles=t1_tiles):
                # kr[op] = yk[op]*cos + (P yk[op])*sin   (1/rms deferred to Exp)
                swb = kpool1.tile([128, cw], bf16, tag=f"swb_{op % 2}")
                nc.scalar.activation(swb[:], ps_sw[:], AF.Identity)
                t2 = kpool1.tile([128, cw], bf16, tag="t2")
                nc.vector.tensor_mul(t2[:], swb[:], stk_t[:])
                nc.vector.tensor_add(kr[:, op, c0:c0 + cw],
                                     t1_tiles.pop(op)[:], t2[:])

            sw_prev = None
            for o in range(8):
                ps_y = pp.tile([128, cw], f32, tag="ps")
                for i in range(8):
                    nc.tensor.matmul(ps_y[:], wk_s[:, i, o, :], memt[:, i, :],
                                     start=(i == 0), stop=(i == 7))
                if sw_prev is not None:
                    # swap matmul for o-1 (after o's projection, so the PE
                    # never waits on yk[o-1]'s activation copy)
                    ps_sw = pp_acc.tile([128, cw], f32, tag="acc")
                    nc.tensor.matmul(ps_sw[:], pt_s[:], yk[:, sw_prev, :])
                    rope_pend = (sw_prev, ps_sw)
                else:
                    rope_pend = None
                nc.scalar.activation(yk[:, o, :], ps_y[:], AF.Identity,
                                     bias=bk_s[:, o:o + 1])
                nc.vector.tensor_mul(ysq_all[:, o, :], yk[:, o, :], yk[:, o, :])
                t1 = kpool1.tile([128, cw], bf16, tag=f"t1_{o % 2}")
                nc.vector.tensor_mul(t1[:], yk[:, o, :], ctk_t[:])
                t1_tiles[o] = t1
                if rope_pend is not None:
                    rope_finish(*rope_pend)
                if last:
                    q_swap_work(o)
                sw_prev = o
            ps_sw = pp_acc.tile([128, cw], f32, tag="acc")
            nc.tensor.matmul(ps_sw[:], pt_s[:], yk[:, 7, :])
            rope_finish(7, ps_sw)
            if last:
                q_finish()

            if pend_sumsq is not None:
                pend_sumsq()

            # V projection (token-major output)
            for ti in range(ntt):
                for oh in range(2):
                    ps_v = pp.tile([128, 512], f32, tag="ps")
                    for i in range(8):
                        nc.tensor.matmul(
                            ps_v[:], memt[:, i, ti * 128:(ti + 1) * 128],
                            wv_s[:, i, oh * 512:(oh + 1) * 512],
                            start=(i == 0), stop=(i == 7))
                    nc.scalar.activation(
                        v_sb[:, ct0 + ti, oh * 512:(oh + 1) * 512], ps_v[:],
                        AF.Identity)
                if last and ti == 1:
                    # last chunk: emit its own sum-sq early so the final Sqrt
                    # (and the Exp-table warm behind it) clears the ACT queue
                    # well before phase 2's first exp
                    make_sumsq(ct0, ntt, ysq_all)()
                    nc.scalar.activation(atl_d[:], eps1_s[:], AF.Exp)

            if not last:
                pend_sumsq = make_sumsq(ct0, ntt, ysq_all)
        qpool_cm.__exit__(None, None, None)
        wqpf_cm.__exit__(None, None, None)
        qpf_cm.__exit__(None, None, None)

        # =========== phase 2: per-head attention, PSUM accumulation ===========
        wopool = pool("wopool", 1)
        wo_s = wopool.tile([128, 8, 8, 128], bf16)
        nc.sync.dma_start(wo_s[:], wo)
        tail = pool("tail", 1)
        nsc = tail.tile([128, 8, QS], bf16)
        ppool_cm = tc.tile_pool(name="ppool", bufs=4)
        ppool = ppool_cm.__enter__()
        p2_cm = tc.tile_pool(name="p2", bufs=2)
        p2 = p2_cm.__enter__()
        p1_cm = tc.tile_pool(name="p1", bufs=1)
        p1 = p1_cm.__enter__()

        def rs_and_normalize(cat_h, catrs_h, hb):
            # reduce-scatter one 4-head half and normalize its numerators;
            # for the first half this overlaps the second half's attention
            if _sim:
                (nc.scalar if hb else nc.gpsimd).dma_start(
                    catrs_h[:], cat_h[0, :, :])
            else:
                nc.gpsimd.collective_compute(
                    "ReduceScatter", mybir.AluOpType.add,
                    replica_groups=[list(range(N_CORES))],
                    ins=[cat_h[:]], outs=[catrs_h[:]])
            dden = tail.tile([1, 4, QS], f32, tag=f"dden{hb}")
            nc.scalar.dma_start(dden[:], catrs_h[DIM // 2:HROWS, :])
            nred = tail.tile([128, 4, QS], f32, tag=f"nred{hb}")
            nc.scalar.dma_start(
                nred[:], catrs_h[0:DIM // 2, :].rearrange("(h p) q -> p h q", p=128))
            ddb = tail.tile([128, 4, QS], f32, tag=f"ddb{hb}")
            nc.gpsimd.partition_broadcast(ddb[:], dden[:])
            nc.vector.reciprocal(ddb[:], ddb[:])
            nc.vector.tensor_mul(nsc[:, 4 * hb:4 * hb + 4, :], nred[:], ddb[:])

        LOOK = 3
        # NOTE: moving den accumulation off the PE (e.g. to the DVE) makes
        # phase 2 ACT-paced with ~160ns PE gaps per tile; each gap resets the
        # PE p-state to half clock, costing far more than the matmuls saved.
        den_split = False
        for h in range(8):
            ps_n = pp_acc.tile([128, SQ], f32, tag="acc")
            den_d = pp_den.tile([1, SQ], f32, tag="den")
            if den_split:
                dacc = p1.tile([128, SQ], f32, tag="dacc")
            else:
                dacc = None
            pts = {}

            def consume(t, h=h, ps_n=ps_n, den_d=den_d, dacc=dacc, pts=pts):
                nc.tensor.matmul(ps_n[:], v_sb[:, t, h * 128:(h + 1) * 128],
                                 pts[t][:], start=(t == 0), stop=(t == tt - 1))
                # denominator: even tiles on the PE (PSUM row), odd tiles on
                # the otherwise-idle DVE; combined by a final f32 matmul.
                # This drops phase 2 from PE-paced to ACT(exp)-paced.
                if den_split and t % 2 == 1:
                    if t == 1:
                        nc.vector.tensor_copy(dacc[:], pts[t][:])
                    else:
                        nc.vector.tensor_add(dacc[:], dacc[:], pts[t][:])
                else:
                    nc.tensor.matmul(den_d[:], ones_s[:], pts[t][:],
                                     start=(t == 0),
                                     stop=(not den_split and t == tt - 1))
                del pts[t]

            for t in range(tt):
                ps_s = pp.tile([128, SQ], f32, tag="ps")
                nc.tensor.matmul(ps_s[:], kr[:, h, t * 128:(t + 1) * 128],
                                 qT[:, h, :])
                pt = ppool.tile([128, SQ], bf16, tag="pt")
                nc.scalar.activation(pt[:], ps_s[:], AF.Exp,
                                     bias=mb_s[:, t:t + 1],
                                     scale=rsb_all[:, t:t + 1])
                pts[t] = pt
                if t >= LOOK:
                    consume(t - LOOK)
            for t in range(max(0, tt - LOOK), tt):
                consume(t)
            if den_split:
                nc.tensor.matmul(den_d[:], ones_fs[:], dacc[:],
                                 start=False, stop=True)

            cat_h = cat_a if h < 4 else cat_b
            hh = h % 4
            dsb = p2.tile([1, SQ], f32, tag="dsb")
            nc.vector.tensor_copy(dsb[:], den_d[:])
            nsb = p2.tile([128, SQ], f32, tag="nsb")
            nc.vector.tensor_copy(nsb[:], ps_n[:])
            nc.scalar.dma_start(
                cat_h[:, hh * 128:(hh + 1) * 128, :].rearrange("b p q -> p b q"),
                nsb[:])
            (nc.scalar if h == 7 else nc.gpsimd).dma_start(
                cat_h[:, DIM // 2 + hh:DIM // 2 + hh + 1, :].rearrange(
                    "b o q -> o b q"),
                dsb[:])
            if h == 3:
                rs_and_normalize(cat_a, catrs_a, 0)
        p1_cm.__exit__(None, None, None)
        p2_cm.__exit__(None, None, None)
        ppool_cm.__exit__(None, None, None)

        # =========== second-half reduce-scatter + output projection ===========
        if _dbg:
            nc.sync.dma_start(qdbg, qT[:])
            nc.sync.dma_start(krdbg, kr[:])
            nc.sync.dma_start(rsbdbg, rsb_all[:])
        rs_and_normalize(cat_b, catrs_b, 1)
        out_sb = tail.tile([128, 8, QS], f32)
        for e in range(8):
            ps_o = pp.tile([128, QS], f32, tag="ps")
            for o in range(8):
                nc.tensor.matmul(ps_o[:], wo_s[:, o, e, :], nsc[:, o, :],
                                 start=(o == 0), stop=(o == 7))
            nc.scalar.activation(out_sb[:, e, :], ps_o[:], AF.Identity,
                                 bias=bo_s[:, e:e + 1])
        if _dbg:
            nscf = tail.tile([128, 8, QS], f32)
            nc.vector.tensor_copy(nscf[:], nsc[:])
            nc.sync.dma_start(nscdbg, nscf[:])
        outr = outT.rearrange("(e p) q -> p e q", p=128)
        nc.sync.dma_start(outr[:, 0:4, :], out_sb[:, 0:4, :])
        nc.sync.dma_start(outr[:, 4:8, :], out_sb[:, 4:8, :])

        for p in reversed(ctx_pools):
            p.__exit__(None, None, None)

    nc.compile()
    _cache[key] = nc
    return nc


def _prep(x, mem, mask, cos_q, sin_q, cos_k, sin_k,
          Wq, bq, Wk, bk, Wv, bv, Wo, bo, gq, gk):
    import ml_dtypes
    f = np.float32
    bf = ml_dtypes.bfloat16
    x = np.asarray(x, f).reshape(SQ, DIM)
    mem = np.asarray(mem, f)
    mem = mem.reshape(-1, DIM)
    sk = mem.shape[0]
    mask = np.asarray(mask).reshape(sk)
    cos_q = np.asarray(cos_q, f)
    sin_q = np.asarray(sin_q, f)
    cos_k = np.asarray(cos_k, f)
    sin_k = np.asarray(sin_k, f)
    Wq, Wk, Wv, Wo = (np.asarray(w, f) for w in (Wq, Wk, Wv, Wo))
    bq, bk, bv, bo, gq, gk = (np.asarray(v, f) for v in (bq, bk, bv, bo, gq, gk))

    assert np.all(bv == 0.0), "nonzero bv not supported by this build"
    if not np.allclose(gk, 1.0):
        gkp = gk.reshape(-1, 2)
        assert np.allclose(gkp[:, 0], gkp[:, 1]), "unsupported non-pairwise gk"

    def tile_w(WT):  # [1024,1024] (in,out of W.T) -> [p, i, o, m]
        return np.ascontiguousarray(
            WT.reshape(8, 128, 8, 128).transpose(1, 0, 2, 3)).astype(bf)

    ii = np.arange(128)
    jj = ii // 2
    partner = ii ^ 1

    # fold gq (+pairwise gk) and the 1/sqrt(HD) score scale into the q rope
    # tables; sin pairs with partner's gq
    gq_t = (gq * gk).reshape(8, 128) * SCALE
    gq_sin = (gq.reshape(8, 128)[:, partner] * gk.reshape(8, 128)) * SCALE
    cq = cos_q[:, jj].T                # [128, SQ]
    sq = sin_q[:, jj].T
    ctq = np.ascontiguousarray(
        (cq[None, :, :] * gq_t[:, :, None]).transpose(1, 0, 2)).astype(bf)
    stq = np.ascontiguousarray(
        (sq[None, :, :] * gq_sin[:, :, None]).transpose(1, 0, 2)).astype(bf)

    PT = np.zeros((128, 128), f)
    even = ii[ii % 2 == 0]
    PT[even + 1, even] = -1.0
    PT[even, even + 1] = 1.0

    shared = {
        "xT": np.ascontiguousarray(x.T).astype(bf),
        "wq": np.ascontiguousarray(tile_w(Wq.T).transpose(2, 0, 1, 3)),
        "wk": tile_w(Wk.T), "wo": tile_w(Wo.T),
        "wv": np.ascontiguousarray(
            Wv.T.reshape(8, 128, DIM).transpose(1, 0, 2)).astype(bf),
        "ctq": ctq, "stq": stq,
        "bq_t": np.ascontiguousarray(bq.reshape(8, 128).T),
        "bk_t": np.ascontiguousarray(bk.reshape(8, 128).T),
        "bo_t": np.ascontiguousarray(bo.reshape(8, 128).T),
        "pmat": PT.astype(bf),
        "ones_c": np.ones((128, 1), bf),
        "ones_fc": np.ones((128, 1), f),
        "eps_c": np.full((128, 1), EPS, f),
        "eps1_c": np.full((1, 1), EPS, f),
    }

    # ---- mask compaction: keep only unmasked keys, pad to 8*tt*128 ----
    idx = np.flatnonzero(mask)
    m = idx.size
    tt = max(1, -(-m // (N_CORES * 128)))
    skc = tt * 128
    total = N_CORES * skc

    memc = np.zeros((total, DIM), f)
    memc[:m] = mem[idx]
    ck = np.zeros((total, HD // 2), f)
    sk_ = np.zeros((total, HD // 2), f)
    ck[:m] = cos_k[idx]
    sk_[:m] = sin_k[idx]
    mb_full = np.full(total, NEG, f)
    mb_full[:m] = 0.0

    in_maps = []
    for c in range(N_CORES):
        s = slice(c * skc, (c + 1) * skc)
        mmap = dict(shared)
        mmap["memT"] = np.ascontiguousarray(memc[s].T).astype(bf)
        mmap["ctk"] = np.ascontiguousarray(ck[s][:, jj].T).astype(bf)
        mmap["stk"] = np.ascontiguousarray(sk_[s][:, jj].T).astype(bf)
        mmap["mbias"] = np.ascontiguousarray(mb_full[s].reshape(tt, 128).T)
        in_maps.append(mmap)
    return in_maps, tt


def kernel(**inputs):
    global _last_tt
    from concourse.bass_utils import run_bass_kernel_spmd
    in_maps, tt = _prep(**inputs)
    _last_tt = tt
    nc = _build(tt)
    res = run_bass_kernel_spmd(nc, in_maps, list(range(N_CORES)))
    parts = [res.results[c]["outT"].T for c in range(N_CORES)]
    out = np.concatenate(parts, axis=0)
    return out[None].astype(np.float32)
